# revision 3
# baseline (speedup 1.0000x reference)
"""Multi-head attention (B=4, S=2048, D=1024, 16 heads) on 8 TRN2 NeuronCores.

Sharding: data-parallel over batch (4) x tensor-parallel over heads (2 groups
of 8).  Core c handles batch c//2, head-group c%2; the host sums the two
partial output projections per batch and adds bo.

v2 design notes (vs the v1 baseline):
  - All input layout work moved to the host: x^T, mask-complement^T and all
    weights arrive in bf16, so the on-device PE transposes, int32 mask
    bounce through DRAM, and fp32->bf16 casts are gone entirely.
    Per-core HBM traffic drops from ~76 MB to ~35 MB.
  - Scores (contraction dk=64 < 128) are issued as two row-tiled matmuls
    (tile_position (0,0) / (64,0)) writing different PSUM banks, which run
    concurrently on the 16x 32x32 sub-arrays -> ~2x effective score rate.
  - kb-granular inner loop: one [128,1024] score tile per (head-pair,
    k-block) covering both heads; single exp per tile on ACT (the true
    critical engine at ~255us); DVE does only the mask multiply (2x mode)
    and PSUM evacuations.
  - PV keeps the transposed (out^T = V_aug^T P^T) dataflow: the natural
    orientation would make pp the 128-column stationary operand and be
    LDWEIGHTS-bound on real hardware.
  - Projections stream x^T in [128,512] column slices (small rotating pool);
    V-projection and the Q-projections for q-spans 1..3 are threaded through
    the first attention sweep so ACT starts exp'ing ~40us in.
"""

import sys

if "/opt/trn_rl_repo" not in sys.path:
    sys.path.insert(0, "/opt/trn_rl_repo")

from contextlib import ExitStack

import numpy as np

import concourse.bass as bass
import concourse.tile as tile
from concourse import mybir

FP32 = mybir.dt.float32
BF16 = mybir.dt.bfloat16

S = 2048
D = 1024
DML = 512          # local d_model slice = 8 heads * 64
DK = 64
HG = DML // DK     # 8 local heads
NPAIR = HG // 2    # 4 head pairs
P = 128
nD = D // P        # 8
nDM = DML // P     # 4
nS = S // P        # 16
SQB = 512          # q-span per attention block
nSQB = S // SQB    # 4
VST = DK + 2       # V_aug per-head stride (64 data + ones + pad)
N_CORES = 8


def build_attention(tc: tile.TileContext, io):
    nc = tc.nc
    ctx = ExitStack()

    with ctx:
        singles = ctx.enter_context(tc.tile_pool(name="singles", bufs=1))
        w_pool = ctx.enter_context(tc.tile_pool(name="w", bufs=1))
        xs_pool = ctx.enter_context(tc.tile_pool(name="xs", bufs=3 * nD + 4))
        xv_pool = ctx.enter_context(tc.tile_pool(name="xv", bufs=1))
        qt_pool = ctx.enter_context(tc.tile_pool(name="qt", bufs=1))
        kt_pool = ctx.enter_context(tc.tile_pool(name="kt", bufs=1))
        va_pool = ctx.enter_context(tc.tile_pool(name="va", bufs=1))
        mt_pool = ctx.enter_context(tc.tile_pool(name="mt", bufs=2 * nS))
        pp_pool = ctx.enter_context(tc.tile_pool(name="pp", bufs=6))
        rd_pool = ctx.enter_context(tc.tile_pool(name="rd", bufs=4))
        xo_pool = ctx.enter_context(tc.tile_pool(name="xo", bufs=2 * nDM))
        ob_pool = ctx.enter_context(tc.tile_pool(name="ob", bufs=4))

        sc_psum = ctx.enter_context(tc.tile_pool(name="sc_ps", bufs=2, space="PSUM"))
        pv_psum = ctx.enter_context(tc.tile_pool(name="pv_ps", bufs=2, space="PSUM"))
        m_psum = ctx.enter_context(tc.tile_pool(name="m_ps", bufs=2, space="PSUM"))

        # ---------------- constants / weights ----------------
        # K path first so the first score matmul is unblocked ASAP
        ones_row = singles.tile([1, SQB], BF16)
        nc.vector.memset(ones_row, 1.0)

        def load_w(name, width):
            ts = []
            n = {"wq": nD, "wk": nD, "wv": nD, "wo": nDM}[name]
            for kj in range(n):
                t = w_pool.tile([P, width], BF16, tag=f"{name}{kj}", name=f"{name}{kj}")
                nc.sync.dma_start(out=t, in_=io[name][kj * P:(kj + 1) * P, :])
                ts.append(t)
            return ts

        def load_bias_col(name):
            # [DML] fp32 dram vector -> [128, nDM] column layout
            # (per-partition scalars for the dm blocks)
            t = singles.tile([P, nDM], FP32, name=f"{name}_col")
            src = bass.AP(tensor=io[name].tensor, offset=io[name].offset,
                          ap=[[1, P], [P, nDM]])
            nc.sync.dma_start(out=t, in_=src)
            return t

        w = {}
        b_col = {}
        w["wk"] = load_w("wk", DML)
        b_col["bk"] = load_bias_col("bk")

        # Q^T/K^T projection, one s-span nb, split into loads + per-mj-pair
        # compute pieces so it can thread through attention in ~1.7us units.
        #   dst[mj][dk-rows, nb-span] = (W^T x^T + b) in bf16
        #   bias folded into the PSUM evacuation as a per-partition add
        def proj_qk_load(nb, x_dram):
            xs = []
            for kj in range(nD):
                t = xs_pool.tile([P, SQB], BF16, tag="xs")
                nc.sync.dma_start(
                    out=t, in_=x_dram[kj * P:(kj + 1) * P, nb * SQB:(nb + 1) * SQB])
                xs.append(t)
            return xs

        def proj_qk_mjs(nb, wname, bname, dst, xs, mjs):
            for mj in mjs:
                ps = m_psum.tile([P, SQB], FP32, tag="m")
                for kj in range(nD):
                    nc.tensor.matmul(ps, w[wname][kj][:, mj * P:(mj + 1) * P],
                                     xs[kj], start=(kj == 0), stop=(kj == nD - 1))
                nc.vector.tensor_scalar(
                    out=dst[mj][:, nb * SQB:(nb + 1) * SQB], in0=ps,
                    scalar1=b_col[bname][:, mj:mj + 1], scalar2=None,
                    op0=mybir.AluOpType.add)

        def proj_qk_nb(nb, x_dram, wname, bname, dst):
            xs = proj_qk_load(nb, x_dram)
            proj_qk_mjs(nb, wname, bname, dst, xs, range(nDM))

        # V projection for one s-block si from the resident xvt tiles.
        # bv is folded into bo on the host (bv@Wo is a constant vector).
        def proj_v(si, va):
            ps = m_psum.tile([P, DML], FP32, tag="m")
            c, r = divmod(si, nSQB)
            for kj in range(nD):
                nc.tensor.matmul(ps, xvc[c][kj][:, r * P:(r + 1) * P], w["wv"][kj],
                                 start=(kj == 0), stop=(kj == nD - 1))
            va3 = va[si].rearrange("p (h e) -> p h e", e=VST)
            ps3 = ps.rearrange("p (h e) -> p h e", e=DK)
            nc.vector.tensor_copy(out=va3[:, :, 0:DK], in_=ps3)
            nc.vector.memset(va3[:, :, DK:DK + 1], 1.0)

        def load_mask(j):
            mts = []
            for kb in range(nS):
                t = mt_pool.tile([P, SQB], BF16, tag="mt", name=f"mt{j}_{kb}")
                nc.sync.dma_start(
                    out=t, in_=io["mct"][kb * P:(kb + 1) * P, j * SQB:(j + 1) * SQB])
                mts.append(t)
            return mts

        qt = [qt_pool.tile([P, S], BF16, tag=f"qt{m}", name=f"qt{m}") for m in range(nDM)]
        kt = [kt_pool.tile([P, S], BF16, tag=f"kt{m}", name=f"kt{m}") for m in range(nDM)]
        va = [va_pool.tile([P, HG * VST], BF16, tag=f"va{si}", name=f"va{si}")
              for si in range(nS)]

        # one attention inner step: scores + exp + mask + PV for (j, pr, kb)
        def attn_step(j, pr, kb, pv, mts, interleave=None):
            sc = sc_psum.tile([P, 2 * SQB], FP32, tag="sc", name=f"sc{j}_{pr}_{kb}")
            for hh in (0, 1):
                nc.tensor.matmul(
                    sc[:, hh * SQB:(hh + 1) * SQB],
                    kt[pr][hh * DK:(hh + 1) * DK, kb * P:(kb + 1) * P],
                    qt[pr][hh * DK:(hh + 1) * DK, j * SQB:(j + 1) * SQB],
                    start=True, stop=True, tile_position=(hh * DK, 0))
            if interleave is not None:
                interleave()
            pp = pp_pool.tile([P, 2 * SQB], BF16, tag="pp", name=f"pp{j}_{pr}_{kb}")
            nc.scalar.activation(pp, sc, mybir.ActivationFunctionType.Exp,
                                 scale=1.0 / np.sqrt(DK))
            # one mask multiply for both heads: mask tile broadcast across the
            # two q-span halves via a 0-stride middle dim
            mt = mts[kb]
            mt3 = bass.AP(tensor=mt.tensor, offset=mt.offset,
                          ap=[list(mt.ap[0]), [0, 2], list(mt.ap[1])])
            pp3 = pp.rearrange("p (t f) -> p t f", t=2)
            nc.vector.tensor_tensor(out=pp3, in0=pp3, in1=mt3,
                                    op=mybir.AluOpType.mult)
            for hh in (0, 1):
                h = 2 * pr + hh
                nc.tensor.matmul(
                    pv[hh][0:DK + 1, :],
                    va[kb][:, h * VST:h * VST + DK + 1],
                    pp[:, hh * SQB:(hh + 1) * SQB],
                    start=(kb == 0), stop=(kb == nS - 1))

        # normalize pv -> xo[dm, q] slices for head-pair pr.  The stt reads
        # both pv and the broadcast reciprocal directly from PSUM.
        def attn_norm(j, pr, pv, xo):
            for hh in (0, 1):
                rden = rd_pool.tile([1, SQB], BF16, tag="rden")
                with nc.allow_low_precision(reason="softmax rdenom bcast in bf16"):
                    nc.vector.reciprocal(rden, pv[hh][DK:DK + 1, :])
                rp = m_psum.tile([P, SQB], FP32, tag="m")
                nc.tensor.matmul(rp[0:DK, :], ones_row[:, 0:DK], rden,
                                 start=True, stop=True)
                rdb = rd_pool.tile([DK, SQB], BF16, tag="rdb")
                nc.vector.tensor_copy(out=rdb, in_=rp[0:DK, :])
                h = 2 * pr + hh
                nc.vector.scalar_tensor_tensor(
                    out=xo[h // 2][(h % 2) * DK:(h % 2 + 1) * DK, :],
                    in0=pv[hh][0:DK, :], scalar=1.0, in1=rdb,
                    op0=mybir.AluOpType.bypass, op1=mybir.AluOpType.mult)

        # output projection for one q-span j from xo[dm, q] tiles
        def out_proj(j, xo):
            for qc in range(SQB // P):
                for nb in range(D // SQB):
                    wp = m_psum.tile([P, SQB], FP32, tag="m")
                    for kj in range(nDM):
                        nc.tensor.matmul(
                            wp, xo[kj][:, qc * P:(qc + 1) * P],
                            wo[kj][:, nb * SQB:(nb + 1) * SQB],
                            start=(kj == 0), stop=(kj == nDM - 1))
                    ob = ob_pool.tile([P, SQB], BF16, tag="ob")
                    nc.vector.tensor_copy(out=ob, in_=wp)
                    nc.sync.dma_start(
                        out=io["out"][j * SQB + qc * P:j * SQB + (qc + 1) * P,
                                      nb * SQB:(nb + 1) * SQB],
                        in_=ob)

        # ---------------- emission ----------------
        # prefix: K + Q for s/q-span 0 only, so the first exp lands ~20us in.
        # K(nb=1..3), Q(nb=1..3) and the V-projection all thread through the
        # first attention sweep in <=1.7us pieces; out_proj(j) is deferred
        # past the first steps of sweep j+1 so ACT never waits at j edges.
        proj_qk_nb(0, io["xkt"], "wk", "bk", kt)
        w["wq"] = load_w("wq", DML)
        b_col["bq"] = load_bias_col("bq")
        proj_qk_nb(0, io["xqt"], "wq", "bq", qt)
        xsk = {}
        xsk[1] = proj_qk_load(1, io["xkt"])
        w["wv"] = load_w("wv", DML)
        # xvt prefetch in column chunks (so V(si) only waits for its chunk),
        # interleaved with the j0 mask tiles so neither starves
        xvc = [[None] * nD for _ in range(nSQB)]
        mts0 = []
        for c in range(nSQB):
            for kj in range(nD):
                t = xv_pool.tile([P, SQB], BF16, tag=f"xv{c}_{kj}",
                                 name=f"xv{c}_{kj}")
                nc.sync.dma_start(
                    out=t, in_=io["xvt"][kj * P:(kj + 1) * P,
                                         c * SQB:(c + 1) * SQB])
                xvc[c][kj] = t
            if c + 2 <= 3:
                xsk[c + 2] = proj_qk_load(c + 2, io["xkt"])
            for kb in (4 * c, 4 * c + 1, 4 * c + 2, 4 * c + 3):
                t2 = mt_pool.tile([P, SQB], BF16, tag="mt", name=f"mt0_{kb}")
                nc.sync.dma_start(
                    out=t2, in_=io["mct"][kb * P:(kb + 1) * P, 0:SQB])
                mts0.append(t2)
        mts_all = {0: mts0}
        wo = []

        # slot -> list of closures, keyed (pr, kb), all within j=0.
        # K(nb) must complete before (pr0, kb=4nb); Q(nb) before sweep j=nb.
        extras = {}

        def sched(pr, kb, fn):
            extras.setdefault((pr, kb), []).append(fn)

        # K(nb) compute pieces land >=1 slot before the sc that consumes
        # kt[:, nb-span] (kb=4nb); the x-slices were loaded in the prefix
        sched(0, 2, lambda: proj_qk_mjs(1, "wk", "bk", kt, xsk[1], (0, 1)))
        sched(0, 3, lambda: proj_qk_mjs(1, "wk", "bk", kt, xsk[1], (2, 3)))
        sched(0, 5, lambda: proj_qk_mjs(2, "wk", "bk", kt, xsk[2], (0, 1)))
        sched(0, 6, lambda: proj_qk_mjs(2, "wk", "bk", kt, xsk[2], (2, 3)))
        sched(0, 9, lambda: proj_qk_mjs(3, "wk", "bk", kt, xsk[3], (0, 1)))
        sched(0, 10, lambda: proj_qk_mjs(3, "wk", "bk", kt, xsk[3], (2, 3)))
        sched(1, 12, lambda: wo.extend(load_w("wo", D)))
        for nbq in (1, 2, 3):
            sched(nbq, 0, lambda nbq=nbq: xsk.__setitem__(
                -nbq, proj_qk_load(nbq, io["xqt"])))
            sched(nbq, 1, lambda nbq=nbq: proj_qk_mjs(
                nbq, "wq", "bq", qt, xsk[-nbq], (0, 1)))
            sched(nbq, 2, lambda nbq=nbq: proj_qk_mjs(
                nbq, "wq", "bq", qt, xsk[-nbq], (2, 3)))

        pending_outproj = None
        for j in range(nSQB):
            mts = mts_all.pop(j)
            if j + 1 < nSQB:
                mts_all[j + 1] = load_mask(j + 1)
            xo = [xo_pool.tile([P, SQB], BF16, tag="xo", name=f"xo{j}_{m}")
                  for m in range(nDM)]
            for pr in range(NPAIR):
                pv = [pv_psum.tile([P, SQB], FP32, tag="pv", name=f"pv{j}_{pr}_{hh}")
                      for hh in (0, 1)]
                for kb in range(nS):
                    fns = []
                    if j == 0:
                        if pr == 0:
                            fns.append(lambda si=kb: proj_v(si, va))
                        fns.extend(extras.pop((pr, kb), []))
                    if pending_outproj is not None and pr == 0 and kb == 4:
                        fns.append(pending_outproj)
                        pending_outproj = None
                    interleave = None
                    if fns:
                        def interleave(fns=fns):
                            for f in fns:
                                f()
                    attn_step(j, pr, kb, pv, mts, interleave)
                attn_norm(j, pr, pv, xo)
            pending_outproj = lambda j=j, xo=xo: out_proj(j, xo)
        pending_outproj()


def split_excess_waits(nc, default_limit=1, drain_limit=1, dma_limit=1):
    """The walrus build accepts at most one semaphore wait per instruction;
    hoist excess waits onto same-engine NoOp carriers."""
    n_new = 0
    for f in nc.m.functions:
        for blk in f.blocks:
            insts = blk.instructions
            pos = 0
            while pos < len(insts):
                i = insts[pos]
                if isinstance(i, mybir.InstDrain):
                    limit = drain_limit
                elif isinstance(i, (mybir.InstDMACopy, mybir.InstDmaTransposeAnt)):
                    limit = dma_limit
                else:
                    limit = default_limit
                si = getattr(i, "sync_info", None)
                if si is not None and si.on_wait is not None and len(si.on_wait) > limit:
                    excess = []
                    while len(si.on_wait) > limit:
                        excess.append(si.on_wait.pop())
                    carriers = []
                    for jj in range(0, len(excess), max(default_limit, 1)):
                        nd = mybir.InstNoOp(name=f"I-sw{n_new}", ins=[], outs=[])
                        n_new += 1
                        nd.engine = i.engine
                        nd.sync_info = mybir.SyncInfo(
                            on_wait=excess[jj:jj + default_limit], on_update=[])
                        carriers.append(nd)
                    for k, nd in enumerate(carriers):
                        insts.insert(pos + k, nd)
                    pos += len(carriers)
                pos += 1
    return n_new


def build_nc(reps=1):
    nc = bass.Bass("TRN2", target_bir_lowering=False, debug=False, num_devices=N_CORES)
    io = {
        "xqt": nc.dram_tensor("xqt", [D, S], BF16, kind="ExternalInput")[:],
        "xkt": nc.dram_tensor("xkt", [D, S], BF16, kind="ExternalInput")[:],
        "xvt": nc.dram_tensor("xvt", [D, S], BF16, kind="ExternalInput")[:],
        "mct": nc.dram_tensor("mct", [S, S], BF16, kind="ExternalInput")[:],
        "wq": nc.dram_tensor("wq", [D, DML], BF16, kind="ExternalInput")[:],
        "wk": nc.dram_tensor("wk", [D, DML], BF16, kind="ExternalInput")[:],
        "wv": nc.dram_tensor("wv", [D, DML], BF16, kind="ExternalInput")[:],
        "wo": nc.dram_tensor("wo", [DML, D], BF16, kind="ExternalInput")[:],
        "bq": nc.dram_tensor("bq", [DML], FP32, kind="ExternalInput")[:],
        "bk": nc.dram_tensor("bk", [DML], FP32, kind="ExternalInput")[:],
        "out": nc.dram_tensor("out", [S, D], BF16, kind="ExternalOutput")[:],
    }
    with tile.TileContext(nc) as tc:
        for _ in range(reps):
            build_attention(tc, io)
    split_excess_waits(nc)
    return nc


def host_in_maps(inputs):
    import ml_dtypes
    bf16 = ml_dtypes.bfloat16
    query = np.asarray(inputs["query"], np.float32)
    key = np.asarray(inputs["key"], np.float32)
    value = np.asarray(inputs["value"], np.float32)
    mask = np.asarray(inputs["mask"], np.int32)
    Wq, bq = np.asarray(inputs["Wq"], np.float32), np.asarray(inputs["bq"], np.float32)
    Wk, bk = np.asarray(inputs["Wk"], np.float32), np.asarray(inputs["bk"], np.float32)
    Wv, bv = np.asarray(inputs["Wv"], np.float32), np.asarray(inputs["bv"], np.float32)
    Wo = np.asarray(inputs["Wo"], np.float32)

    in_maps = []
    for c in range(N_CORES):
        b, g = divmod(c, 2)
        sl = slice(g * DML, (g + 1) * DML)
        mct = (mask[b].T == 0).astype(bf16)
        in_maps.append({
            "xqt": np.ascontiguousarray(query[b].T.astype(bf16)),
            "xkt": np.ascontiguousarray(key[b].T.astype(bf16)),
            "xvt": np.ascontiguousarray(value[b].T.astype(bf16)),
            "mct": np.ascontiguousarray(mct),
            "wq": np.ascontiguousarray(Wq[:, sl].astype(bf16)),
            "wk": np.ascontiguousarray(Wk[:, sl].astype(bf16)),
            "wv": np.ascontiguousarray(Wv[:, sl].astype(bf16)),
            "wo": np.ascontiguousarray(Wo[sl, :].astype(bf16)),
            "bq": np.ascontiguousarray(bq[sl]),
            "bk": np.ascontiguousarray(bk[sl]),
        })
    return in_maps


_NC_CACHE = {}


def kernel(**inputs):
    # bv@Wo is a constant vector: fold it into bo on the host (exact in fp32)
    bo = (np.asarray(inputs["bo"], np.float32)
          + np.asarray(inputs["bv"], np.float32) @ np.asarray(inputs["Wo"], np.float32))
    B = np.asarray(inputs["query"]).shape[0]

    if "nc" not in _NC_CACHE:
        _NC_CACHE["nc"] = build_nc()
    nc = _NC_CACHE["nc"]

    in_maps = host_in_maps(inputs)

    from concourse.bass_utils import run_bass_kernel_spmd
    res = run_bass_kernel_spmd(nc, in_maps, core_ids=list(range(N_CORES)))
    out = np.stack([
        res.results[2 * b]["out"].astype(np.float32)
        + res.results[2 * b + 1]["out"].astype(np.float32) + bo
        for b in range(B)
    ]).astype(np.float32)
    return out


# revision 4
# speedup vs baseline: 11.4391x; 11.4391x over previous
"""Multi-head attention (B=4, S=2048, D=1024, 16 heads) on 8 TRN2 NeuronCores.

Sharding: data-parallel over batch (4) x tensor-parallel over heads (2 groups
of 8).  Core c handles batch c//2, head-group c%2; the host sums the two
partial output projections per batch and adds bo.

v2 design notes (vs the v1 baseline):
  - All input layout work moved to the host: x^T, mask-complement^T and all
    weights arrive in bf16, so the on-device PE transposes, int32 mask
    bounce through DRAM, and fp32->bf16 casts are gone entirely.
    Per-core HBM traffic drops from ~76 MB to ~35 MB.
  - Scores (contraction dk=64 < 128) are issued as two row-tiled matmuls
    (tile_position (0,0) / (64,0)) writing different PSUM banks, which run
    concurrently on the 16x 32x32 sub-arrays -> ~2x effective score rate.
  - kb-granular inner loop: one [128,1024] score tile per (head-pair,
    k-block) covering both heads; single exp per tile on ACT (the true
    critical engine at ~255us); DVE does only the mask multiply (2x mode)
    and PSUM evacuations.
  - PV keeps the transposed (out^T = V_aug^T P^T) dataflow: the natural
    orientation would make pp the 128-column stationary operand and be
    LDWEIGHTS-bound on real hardware.
  - Projections stream x^T in [128,512] column slices (small rotating pool);
    V-projection and the Q-projections for q-spans 1..3 are threaded through
    the first attention sweep so ACT starts exp'ing ~40us in.
"""

import sys

if "/opt/trn_rl_repo" not in sys.path:
    sys.path.insert(0, "/opt/trn_rl_repo")

from contextlib import ExitStack

import numpy as np

import concourse.bass as bass
import concourse.tile as tile
from concourse import mybir

FP32 = mybir.dt.float32
BF16 = mybir.dt.bfloat16

S = 2048
D = 1024
DML = 512          # local d_model slice = 8 heads * 64
DK = 64
HG = DML // DK     # 8 local heads
NPAIR = HG // 2    # 4 head pairs
P = 128
nD = D // P        # 8
nDM = DML // P     # 4
nS = S // P        # 16
SQB = 512          # q-span per attention block
nSQB = S // SQB    # 4
VST = DK + 2       # V_aug per-head stride (64 data + ones + pad)
N_CORES = 8


def build_attention(tc: tile.TileContext, io):
    nc = tc.nc
    ctx = ExitStack()

    with ctx:
        singles = ctx.enter_context(tc.tile_pool(name="singles", bufs=1))
        w_pool = ctx.enter_context(tc.tile_pool(name="w", bufs=1))
        xs_pool = ctx.enter_context(tc.tile_pool(name="xs", bufs=3 * nD + 4))
        xv_pool = ctx.enter_context(tc.tile_pool(name="xv", bufs=1))
        qt_pool = ctx.enter_context(tc.tile_pool(name="qt", bufs=1))
        kt_pool = ctx.enter_context(tc.tile_pool(name="kt", bufs=1))
        va_pool = ctx.enter_context(tc.tile_pool(name="va", bufs=1))
        mt_pool = ctx.enter_context(tc.tile_pool(name="mt", bufs=2 * nS))
        pp_pool = ctx.enter_context(tc.tile_pool(name="pp", bufs=6))
        rd_pool = ctx.enter_context(tc.tile_pool(name="rd", bufs=4))
        xo_pool = ctx.enter_context(tc.tile_pool(name="xo", bufs=2 * nDM))
        ob_pool = ctx.enter_context(tc.tile_pool(name="ob", bufs=4))

        sc_psum = ctx.enter_context(tc.tile_pool(name="sc_ps", bufs=2, space="PSUM"))
        pv_psum = ctx.enter_context(tc.tile_pool(name="pv_ps", bufs=2, space="PSUM"))
        m_psum = ctx.enter_context(tc.tile_pool(name="m_ps", bufs=2, space="PSUM"))

        # ---------------- constants / weights ----------------
        # K path first so the first score matmul is unblocked ASAP
        ones_row = singles.tile([1, SQB], BF16)
        nc.vector.memset(ones_row, 1.0)

        def load_w(name, width):
            ts = []
            n = {"wq": nD, "wk": nD, "wv": nD, "wo": nDM}[name]
            for kj in range(n):
                t = w_pool.tile([P, width], BF16, tag=f"{name}{kj}", name=f"{name}{kj}")
                nc.sync.dma_start(out=t, in_=io[name][kj * P:(kj + 1) * P, :])
                ts.append(t)
            return ts

        def load_bias_col(name):
            # [DML] fp32 dram vector -> [128, nDM] column layout
            # (per-partition scalars for the dm blocks)
            t = singles.tile([P, nDM], FP32, name=f"{name}_col")
            src = bass.AP(tensor=io[name].tensor, offset=io[name].offset,
                          ap=[[1, P], [P, nDM]])
            nc.sync.dma_start(out=t, in_=src)
            return t

        w = {}
        b_col = {}

        # Q^T/K^T projection, one s-span nb, split into loads + per-mj-pair
        # compute pieces so it can thread through attention in ~1.7us units.
        #   dst[mj][dk-rows, nb-span] = (W^T x^T + b) in bf16
        #   bias folded into the PSUM evacuation as a per-partition add
        def proj_qk_load(nb, x_dram):
            xs = []
            for kj in range(nD):
                t = xs_pool.tile([P, SQB], BF16, tag="xs")
                nc.sync.dma_start(
                    out=t, in_=x_dram[kj * P:(kj + 1) * P, nb * SQB:(nb + 1) * SQB])
                xs.append(t)
            return xs

        def proj_qk_mjs(nb, wname, bname, dst, xs, mjs):
            for mj in mjs:
                ps = m_psum.tile([P, SQB], FP32, tag="m")
                for kj in range(nD):
                    nc.tensor.matmul(ps, w[wname][kj][:, mj * P:(mj + 1) * P],
                                     xs[kj], start=(kj == 0), stop=(kj == nD - 1))
                nc.vector.tensor_scalar(
                    out=dst[mj][:, nb * SQB:(nb + 1) * SQB], in0=ps,
                    scalar1=b_col[bname][:, mj:mj + 1], scalar2=None,
                    op0=mybir.AluOpType.add)

        def proj_qk_nb(nb, x_dram, wname, bname, dst):
            xs = proj_qk_load(nb, x_dram)
            proj_qk_mjs(nb, wname, bname, dst, xs, range(nDM))

        # V projection for one s-block si from the resident xvt tiles.
        # bv is folded into bo on the host (bv@Wo is a constant vector).
        def proj_v(si, va):
            ps = m_psum.tile([P, DML], FP32, tag="m")
            c, r = divmod(si, nSQB)
            for kj in range(nD):
                nc.tensor.matmul(ps, xvc[c][kj][:, r * P:(r + 1) * P], w["wv"][kj],
                                 start=(kj == 0), stop=(kj == nD - 1))
            va3 = va[si].rearrange("p (h e) -> p h e", e=VST)
            ps3 = ps.rearrange("p (h e) -> p h e", e=DK)
            nc.vector.tensor_copy(out=va3[:, :, 0:DK], in_=ps3)
            nc.vector.memset(va3[:, :, DK:DK + 1], 1.0)

        def load_mask(j):
            mts = []
            for kb in range(nS):
                t = mt_pool.tile([P, SQB], BF16, tag="mt", name=f"mt{j}_{kb}")
                nc.sync.dma_start(
                    out=t, in_=io["mct"][kb * P:(kb + 1) * P, j * SQB:(j + 1) * SQB])
                mts.append(t)
            return mts

        qt = [qt_pool.tile([P, S], BF16, tag=f"qt{m}", name=f"qt{m}") for m in range(nDM)]
        kt = [kt_pool.tile([P, S], BF16, tag=f"kt{m}", name=f"kt{m}") for m in range(nDM)]
        va = [va_pool.tile([P, HG * VST], BF16, tag=f"va{si}", name=f"va{si}")
              for si in range(nS)]

        # one attention inner step: scores + exp + mask + PV for (j, pr, kb)
        def attn_step(j, pr, kb, pv, mts, interleave=None):
            sc = sc_psum.tile([P, 2 * SQB], FP32, tag="sc", name=f"sc{j}_{pr}_{kb}")
            for hh in (0, 1):
                nc.tensor.matmul(
                    sc[:, hh * SQB:(hh + 1) * SQB],
                    kt[pr][hh * DK:(hh + 1) * DK, kb * P:(kb + 1) * P],
                    qt[pr][hh * DK:(hh + 1) * DK, j * SQB:(j + 1) * SQB],
                    start=True, stop=True, tile_position=(hh * DK, 0))
            if interleave is not None:
                interleave()
            pp = pp_pool.tile([P, 2 * SQB], BF16, tag="pp", name=f"pp{j}_{pr}_{kb}")
            nc.scalar.activation(pp, sc, mybir.ActivationFunctionType.Exp,
                                 scale=1.0 / np.sqrt(DK))
            # one mask multiply for both heads: mask tile broadcast across the
            # two q-span halves via a 0-stride middle dim
            mt = mts[kb]
            mt3 = bass.AP(tensor=mt.tensor, offset=mt.offset,
                          ap=[list(mt.ap[0]), [0, 2], list(mt.ap[1])])
            pp3 = pp.rearrange("p (t f) -> p t f", t=2)
            nc.vector.tensor_tensor(out=pp3, in0=pp3, in1=mt3,
                                    op=mybir.AluOpType.mult)
            for hh in (0, 1):
                h = 2 * pr + hh
                nc.tensor.matmul(
                    pv[hh][0:DK + 1, :],
                    va[kb][:, h * VST:h * VST + DK + 1],
                    pp[:, hh * SQB:(hh + 1) * SQB],
                    start=(kb == 0), stop=(kb == nS - 1))

        # normalize pv -> xo[dm, q] slices for head-pair pr.  The stt reads
        # both pv and the broadcast reciprocal directly from PSUM.
        def attn_norm(j, pr, pv, xo):
            for hh in (0, 1):
                rden = rd_pool.tile([1, SQB], BF16, tag="rden")
                with nc.allow_low_precision(reason="softmax rdenom bcast in bf16"):
                    nc.vector.reciprocal(rden, pv[hh][DK:DK + 1, :])
                rp = m_psum.tile([P, SQB], FP32, tag="m")
                nc.tensor.matmul(rp[0:DK, :], ones_row[:, 0:DK], rden,
                                 start=True, stop=True)
                rdb = rd_pool.tile([DK, SQB], BF16, tag="rdb")
                nc.vector.tensor_copy(out=rdb, in_=rp[0:DK, :])
                h = 2 * pr + hh
                nc.vector.scalar_tensor_tensor(
                    out=xo[h // 2][(h % 2) * DK:(h % 2 + 1) * DK, :],
                    in0=pv[hh][0:DK, :], scalar=1.0, in1=rdb,
                    op0=mybir.AluOpType.bypass, op1=mybir.AluOpType.mult)

        # output projection for one q-span j from xo[dm, q] tiles
        def out_proj(j, xo):
            for qc in range(SQB // P):
                for nb in range(D // SQB):
                    wp = m_psum.tile([P, SQB], FP32, tag="m")
                    for kj in range(nDM):
                        nc.tensor.matmul(
                            wp, xo[kj][:, qc * P:(qc + 1) * P],
                            wo[kj][:, nb * SQB:(nb + 1) * SQB],
                            start=(kj == 0), stop=(kj == nDM - 1))
                    ob = ob_pool.tile([P, SQB], BF16, tag="ob")
                    nc.vector.tensor_copy(out=ob, in_=wp)
                    nc.sync.dma_start(
                        out=io["out"][j * SQB + qc * P:j * SQB + (qc + 1) * P,
                                      nb * SQB:(nb + 1) * SQB],
                        in_=ob)

        # ---------------- emission ----------------
        # prefix: K + Q for s/q-span 0 only, so the first exp lands ~20us in.
        # K(nb=1..3), Q(nb=1..3) and the V-projection all thread through the
        # first attention sweep in <=1.7us pieces; out_proj(j) is deferred
        # past the first steps of sweep j+1 so ACT never waits at j edges.
        # wk and the first x^T slices load interleaved so matmul kj can start
        # as soon as its own operands land.
        w["wk"] = []
        xs0 = []
        for kj in range(nD):
            t = w_pool.tile([P, DML], BF16, tag=f"wk{kj}", name=f"wk{kj}")
            nc.sync.dma_start(out=t, in_=io["wk"][kj * P:(kj + 1) * P, :])
            w["wk"].append(t)
            t2 = xs_pool.tile([P, SQB], BF16, tag="xs")
            nc.sync.dma_start(out=t2, in_=io["xkt"][kj * P:(kj + 1) * P, 0:SQB])
            xs0.append(t2)
        b_col["bk"] = load_bias_col("bk")
        proj_qk_mjs(0, "wk", "bk", kt, xs0, range(nDM))
        w["wq"] = load_w("wq", DML)
        b_col["bq"] = load_bias_col("bq")
        proj_qk_nb(0, io["xqt"], "wq", "bq", qt)
        xsk = {}
        xsk[1] = proj_qk_load(1, io["xkt"])
        w["wv"] = load_w("wv", DML)
        # xvt prefetch in column chunks (so V(si) only waits for its chunk),
        # interleaved with the j0 mask tiles so neither starves
        xvc = [[None] * nD for _ in range(nSQB)]
        mts0 = []
        for c in range(nSQB):
            for kj in range(nD):
                t = xv_pool.tile([P, SQB], BF16, tag=f"xv{c}_{kj}",
                                 name=f"xv{c}_{kj}")
                nc.sync.dma_start(
                    out=t, in_=io["xvt"][kj * P:(kj + 1) * P,
                                         c * SQB:(c + 1) * SQB])
                xvc[c][kj] = t
            if c + 2 <= 3:
                xsk[c + 2] = proj_qk_load(c + 2, io["xkt"])
            for kb in (4 * c, 4 * c + 1, 4 * c + 2, 4 * c + 3):
                t2 = mt_pool.tile([P, SQB], BF16, tag="mt", name=f"mt0_{kb}")
                nc.sync.dma_start(
                    out=t2, in_=io["mct"][kb * P:(kb + 1) * P, 0:SQB])
                mts0.append(t2)
        mts_all = {0: mts0}
        wo = []

        # slot -> list of closures, keyed (pr, kb), all within j=0.
        # K(nb) must complete before (pr0, kb=4nb); Q(nb) before sweep j=nb.
        extras = {}

        def sched(pr, kb, fn):
            extras.setdefault((pr, kb), []).append(fn)

        # K(nb) compute pieces land >=1 slot before the sc that consumes
        # kt[:, nb-span] (kb=4nb); the x-slices were loaded in the prefix
        sched(0, 2, lambda: proj_qk_mjs(1, "wk", "bk", kt, xsk[1], (0, 1)))
        sched(0, 3, lambda: proj_qk_mjs(1, "wk", "bk", kt, xsk[1], (2, 3)))
        sched(0, 5, lambda: proj_qk_mjs(2, "wk", "bk", kt, xsk[2], (0, 1)))
        sched(0, 6, lambda: proj_qk_mjs(2, "wk", "bk", kt, xsk[2], (2, 3)))
        sched(0, 9, lambda: proj_qk_mjs(3, "wk", "bk", kt, xsk[3], (0, 1)))
        sched(0, 10, lambda: proj_qk_mjs(3, "wk", "bk", kt, xsk[3], (2, 3)))
        sched(1, 12, lambda: wo.extend(load_w("wo", D)))
        for nbq in (1, 2, 3):
            sched(nbq, 0, lambda nbq=nbq: xsk.__setitem__(
                -nbq, proj_qk_load(nbq, io["xqt"])))
            sched(nbq, 1, lambda nbq=nbq: proj_qk_mjs(
                nbq, "wq", "bq", qt, xsk[-nbq], (0, 1)))
            sched(nbq, 2, lambda nbq=nbq: proj_qk_mjs(
                nbq, "wq", "bq", qt, xsk[-nbq], (2, 3)))

        pending_outproj = None
        for j in range(nSQB):
            mts = mts_all.pop(j)
            if j + 1 < nSQB:
                mts_all[j + 1] = load_mask(j + 1)
            xo = [xo_pool.tile([P, SQB], BF16, tag="xo", name=f"xo{j}_{m}")
                  for m in range(nDM)]
            for pr in range(NPAIR):
                pv = [pv_psum.tile([P, SQB], FP32, tag="pv", name=f"pv{j}_{pr}_{hh}")
                      for hh in (0, 1)]
                for kb in range(nS):
                    fns = []
                    if j == 0:
                        if pr == 0:
                            fns.append(lambda si=kb: proj_v(si, va))
                        fns.extend(extras.pop((pr, kb), []))
                    if pending_outproj is not None and pr == 0 and kb == 4:
                        fns.append(pending_outproj)
                        pending_outproj = None
                    interleave = None
                    if fns:
                        def interleave(fns=fns):
                            for f in fns:
                                f()
                    attn_step(j, pr, kb, pv, mts, interleave)
                attn_norm(j, pr, pv, xo)
            pending_outproj = lambda j=j, xo=xo: out_proj(j, xo)
        pending_outproj()


def split_excess_waits(nc, default_limit=1, drain_limit=1, dma_limit=1):
    """The walrus build accepts at most one semaphore wait per instruction;
    hoist excess waits onto same-engine NoOp carriers."""
    n_new = 0
    for f in nc.m.functions:
        for blk in f.blocks:
            insts = blk.instructions
            pos = 0
            while pos < len(insts):
                i = insts[pos]
                if isinstance(i, mybir.InstDrain):
                    limit = drain_limit
                elif isinstance(i, (mybir.InstDMACopy, mybir.InstDmaTransposeAnt)):
                    limit = dma_limit
                else:
                    limit = default_limit
                si = getattr(i, "sync_info", None)
                if si is not None and si.on_wait is not None and len(si.on_wait) > limit:
                    excess = []
                    while len(si.on_wait) > limit:
                        excess.append(si.on_wait.pop())
                    carriers = []
                    for jj in range(0, len(excess), max(default_limit, 1)):
                        nd = mybir.InstNoOp(name=f"I-sw{n_new}", ins=[], outs=[])
                        n_new += 1
                        nd.engine = i.engine
                        nd.sync_info = mybir.SyncInfo(
                            on_wait=excess[jj:jj + default_limit], on_update=[])
                        carriers.append(nd)
                    for k, nd in enumerate(carriers):
                        insts.insert(pos + k, nd)
                    pos += len(carriers)
                pos += 1
    return n_new


def build_nc(reps=1):
    nc = bass.Bass("TRN2", target_bir_lowering=False, debug=False, num_devices=N_CORES)
    io = {
        "xqt": nc.dram_tensor("xqt", [D, S], BF16, kind="ExternalInput")[:],
        "xkt": nc.dram_tensor("xkt", [D, S], BF16, kind="ExternalInput")[:],
        "xvt": nc.dram_tensor("xvt", [D, S], BF16, kind="ExternalInput")[:],
        "mct": nc.dram_tensor("mct", [S, S], BF16, kind="ExternalInput")[:],
        "wq": nc.dram_tensor("wq", [D, DML], BF16, kind="ExternalInput")[:],
        "wk": nc.dram_tensor("wk", [D, DML], BF16, kind="ExternalInput")[:],
        "wv": nc.dram_tensor("wv", [D, DML], BF16, kind="ExternalInput")[:],
        "wo": nc.dram_tensor("wo", [DML, D], BF16, kind="ExternalInput")[:],
        "bq": nc.dram_tensor("bq", [DML], FP32, kind="ExternalInput")[:],
        "bk": nc.dram_tensor("bk", [DML], FP32, kind="ExternalInput")[:],
        "out": nc.dram_tensor("out", [S, D], BF16, kind="ExternalOutput")[:],
    }
    with tile.TileContext(nc) as tc:
        for _ in range(reps):
            build_attention(tc, io)
    split_excess_waits(nc)
    return nc


def host_in_maps(inputs):
    import ml_dtypes
    bf16 = ml_dtypes.bfloat16
    query = np.asarray(inputs["query"], np.float32)
    key = np.asarray(inputs["key"], np.float32)
    value = np.asarray(inputs["value"], np.float32)
    mask = np.asarray(inputs["mask"], np.int32)
    Wq, bq = np.asarray(inputs["Wq"], np.float32), np.asarray(inputs["bq"], np.float32)
    Wk, bk = np.asarray(inputs["Wk"], np.float32), np.asarray(inputs["bk"], np.float32)
    Wv, bv = np.asarray(inputs["Wv"], np.float32), np.asarray(inputs["bv"], np.float32)
    Wo = np.asarray(inputs["Wo"], np.float32)

    in_maps = []
    for c in range(N_CORES):
        b, g = divmod(c, 2)
        sl = slice(g * DML, (g + 1) * DML)
        mct = (mask[b].T == 0).astype(bf16)
        in_maps.append({
            "xqt": np.ascontiguousarray(query[b].T.astype(bf16)),
            "xkt": np.ascontiguousarray(key[b].T.astype(bf16)),
            "xvt": np.ascontiguousarray(value[b].T.astype(bf16)),
            "mct": np.ascontiguousarray(mct),
            "wq": np.ascontiguousarray(Wq[:, sl].astype(bf16)),
            "wk": np.ascontiguousarray(Wk[:, sl].astype(bf16)),
            "wv": np.ascontiguousarray(Wv[:, sl].astype(bf16)),
            "wo": np.ascontiguousarray(Wo[sl, :].astype(bf16)),
            "bq": np.ascontiguousarray(bq[sl]),
            "bk": np.ascontiguousarray(bk[sl]),
        })
    return in_maps


_NC_CACHE = {}


def kernel(**inputs):
    # bv@Wo is a constant vector: fold it into bo on the host (exact in fp32)
    bo = (np.asarray(inputs["bo"], np.float32)
          + np.asarray(inputs["bv"], np.float32) @ np.asarray(inputs["Wo"], np.float32))
    B = np.asarray(inputs["query"]).shape[0]

    if "nc" not in _NC_CACHE:
        _NC_CACHE["nc"] = build_nc()
    nc = _NC_CACHE["nc"]

    in_maps = host_in_maps(inputs)

    from concourse.bass_utils import run_bass_kernel_spmd
    res = run_bass_kernel_spmd(nc, in_maps, core_ids=list(range(N_CORES)))
    out = np.stack([
        res.results[2 * b]["out"].astype(np.float32)
        + res.results[2 * b + 1]["out"].astype(np.float32) + bo
        for b in range(B)
    ]).astype(np.float32)
    return out
